# revision 15
# baseline (speedup 1.0000x reference)
"""DualPathAttention Trainium2 Bass kernel (v4, fp16 + software pipelining).

Sharding: batch*head parallel across 8 cores. Core c handles batch b=c//4 and
global heads [4*(c%4), 4*(c%4)+4). Each core computes its 4 heads' dual-path
attention and the partial final projection (its 256 rows of out_w); the host
sums the 4 partials per batch and adds out_b.

Per-core structure:
  A1  Pluecker lines: packed projections PAC=[A|C]^T x, PBD=[B|D]^T x (the
      gate weights ride along unused rows), products with the x_prev shift,
      CD half shifted down via SBUF DMA, normalize, scatter per-head slots.
  A2a q^T/k^T as head pairs [128, 2, T] (no padding); std logits K=64.
  A2b v|geo_v in one [T x 512] stream into the U-matmul lhsT layout with a
      ones column (softmax denominators at PSUM row 64 for free).
  B   per (q-block j, head p): nkt std-logits (K=64) + geo-logits (K=6)
      matmuls, exp split ACT/DVE (geo via fp16 Schraudolph int-bitcast),
      causal masks on gpsimd, U accumulation; block-diagonal tiles compute
      only columns >= min(128*m, 256). Combine (alpha=(1-g)/Dstd etc. via
      K=1 ones-broadcast matmuls) is emitted one head late, and phase C one
      q-block late, so the in-order PE queue never stalls on their deps.
  C   comb2 [128, 2, T] packs head pairs: final projection contracts K=128
      with no padding (odd heads reach rows 64:128 via small SBUF DMA).
All matmul storage is fp16 (1 cyc/row like bf16 but ~8x lower rounding
error; every value here is < 6e4 so range is fine). PSUM stays fp32.
Weights are host-pre-arranged to contiguous [128, 8*C] chunk layout and all
weight DMAs issue upfront, overlapped with A1.
"""

import os
import numpy as np

import concourse.bass as bass
from concourse import bacc
import concourse.mybir as mybir
import concourse.tile as tile
from concourse.bass_utils import run_bass_kernel_spmd

D, H, B, T = 1024, 16, 2, 2048
DH = 64          # head dim
NH = 4           # heads per core
NCORES = 8
QB = 512         # q block width
KT = 128         # k tile height
NQB = T // QB    # 4
F32 = mybir.dt.float32
I16 = mybir.dt.int16
F16 = mybir.dt.float16

PAIRS4 = [(0, 1), (0, 2), (0, 3), (1, 2), (1, 3), (2, 3)]
SIGMA = [1.0, -1.0, 1.0, 1.0, -1.0, 1.0]
GS = [64, 0, 96, 32]      # geo line row-slot per local head (opposite half)
GEO_ACT_EVERY = 2         # every Nth geo exp tile runs on ACT instead of DVE
A_SCH = 1477.3197218702985   # 2^10/ln2  (fp16-bitcast Schraudolph exp)
B_SCH = 15360.0              # 15 << 10

TRACE = False            # set by test harness for profiling runs
LAST_RESULT = None       # BassKernelResults of last run (for exec_time_ns)

F16NP = np.float16


def _build_nc():
    nc = bacc.Bacc("TRN2", target_bir_lowering=False, debug=False)

    # ---- DRAM I/O ----
    d_xT = nc.dram_tensor("xT", [D, T], F16, kind="ExternalInput")
    d_wqk = nc.dram_tensor("wqk", [128, 4096], F16, kind="ExternalInput")
    d_bqk = nc.dram_tensor("bqk", [128, 4], F32, kind="ExternalInput")
    d_wvg = nc.dram_tensor("wvg", [128, 4096], F16, kind="ExternalInput")
    d_bvg = nc.dram_tensor("bvg", [1, 512], F16, kind="ExternalInput")
    d_wlAC = nc.dram_tensor("wlAC", [128, 1024], F16, kind="ExternalInput")
    d_wlBD = nc.dram_tensor("wlBD", [128, 1024], F16, kind="ExternalInput")
    d_wgate = nc.dram_tensor("wgate", [128, 128], F16, kind="ExternalInput")
    d_bgate = nc.dram_tensor("bgate", [16, 1], F32, kind="ExternalInput")
    d_gsel = nc.dram_tensor("gsel", [16, 1], F16, kind="ExternalInput")
    d_sbc = nc.dram_tensor("sbc", [64, 1], F32, kind="ExternalInput")
    d_ssel = nc.dram_tensor("ssel", [64, 64], F16, kind="ExternalInput")
    d_ones = nc.dram_tensor("ones", [1, 128], F16, kind="ExternalInput")
    d_outw = nc.dram_tensor("outw", [128, 2 * D], F16, kind="ExternalInput")
    d_partial = nc.dram_tensor("partial", [T, D], F32, kind="ExternalOutput")

    AF = mybir.ActivationFunctionType
    OP = mybir.AluOpType

    with tile.TileContext(nc, linearize=bool(int(os.environ.get('KLIN', '0')))) as tc:
        with (
            tc.tile_pool(name="const", bufs=1) as cpool,
            tc.tile_pool(name="pers1", bufs=1) as pers1,
        ):
            # ---- constants ----
            ones_sb = cpool.tile([1, 128], F16)
            nc.sync.dma_start(ones_sb[:], d_ones[:])
            gsel = cpool.tile([16, 1], F16)
            nc.sync.dma_start(gsel[:], d_gsel[:])
            ssel = cpool.tile([64, 64], F16)
            nc.sync.dma_start(ssel[:], d_ssel[:])
            sbc_sb = cpool.tile([64, 1], F32)
            nc.sync.dma_start(sbc_sb[:], d_sbc[:])
            bgate_sb = cpool.tile([16, 1], F32)
            nc.sync.dma_start(bgate_sb[:], d_bgate[:])

            jwT = pers1.tile([128, T], F16)   # head p lines at rows GS[p]:+6
            rlT = pers1.tile([128, T], F16)
            g_row = pers1.tile([1, T], F32)
            g1m_row = pers1.tile([1, T], F32)
            outw_sb = pers1.tile([128, 2, D], F16)

            pers2_cm = tc.tile_pool(name="pers2", bufs=1, side="right")
            pers3_cm = tc.tile_pool(name="pers3", bufs=1, side="right")
            pers2 = pers2_cm.__enter__()
            pers3 = None
            try:
                qT = pers2.tile([128, 2, T], F16)
                kT = pers2.tile([128, 2, T], F16)
                vplus = pers2.tile([128, 16, 4, 65], F16)
                gvplus = pers2.tile([128, 16, 4, 65], F16)
                nc.vector.memset(vplus[:, :, :, 64:65], 1.0)
                nc.vector.memset(gvplus[:, :, :, 64:65], 1.0)

                with (
                    tc.tile_pool(name="wA", bufs=1) as wA,
                    tc.tile_pool(name="xp", bufs=1) as xp,
                    tc.tile_pool(name="psA", bufs=4,
                                 space=bass.MemorySpace.PSUM) as psA,
                ):
                    xT_sb = xp.tile([128, 8, T], F16)
                    # all weights upfront, contiguous rows (host pre-arranged)
                    wlAC_sb = wA.tile([128, 8, 128], F16)
                    wlBD_sb = wA.tile([128, 8, 128], F16)
                    wqk_sb = wA.tile([128, 8, 512], F16)
                    wvg_sb = wA.tile([128, 8, 512], F16)
                    wgate_sb = wA.tile([128, 8, 16], F16)
                    bqk_sb = wA.tile([128, 4], F32)
                    bvg_sb = wA.tile([1, 512], F16)
                    flat = "p a b -> p (a b)"
                    nc.sync.dma_start(wlAC_sb[:].rearrange(flat), d_wlAC[:])
                    nc.sync.dma_start(wlBD_sb[:].rearrange(flat), d_wlBD[:])
                    nc.sync.dma_start(xT_sb[:, 0, :], d_xT[0:128, :])
                    nc.sync.dma_start(xT_sb[:, 1, :], d_xT[128:256, :])
                    nc.sync.dma_start(wqk_sb[:].rearrange(flat), d_wqk[:])
                    for ko in range(2, 8):
                        nc.sync.dma_start(
                            xT_sb[:, ko, :], d_xT[128 * ko:128 * (ko + 1), :])
                    nc.sync.dma_start(wvg_sb[:].rearrange(flat), d_wvg[:])
                    nc.sync.dma_start(wgate_sb[:].rearrange(flat), d_wgate[:])
                    nc.sync.dma_start(bqk_sb[:], d_bqk[:])
                    nc.sync.dma_start(bvg_sb[:], d_bvg[:])
                    nc.sync.dma_start(outw_sb[:].rearrange(flat), d_outw[:])

                    with tc.tile_pool(name="wl", bufs=1) as wl:
                        # ---------- A1 projections ----------
                        PAC = wl.tile([128, T], F16, tag="pa")
                        PBD = wl.tile([128, T], F16, tag="pb")
                        for (dst, wsb) in ((PAC, wlAC_sb), (PBD, wlBD_sb)):
                            for tb in range(NQB):
                                ps = psA.tile([128, QB], F32, tag="a")
                                for kc in range(8):
                                    nc.tensor.matmul(
                                        ps[:], wsb[:, kc, :],
                                        xT_sb[:, kc, QB * tb:QB * (tb + 1)],
                                        start=(kc == 0), stop=(kc == 7))
                                nc.scalar.copy(dst[:, QB * tb:QB * (tb + 1)],
                                               ps[:])

                        # ---------- A2a q/k (keeps PE busy while the lines
                        # vector chain below runs on DVE/ACT) ----------
                        for half in range(2):           # 0: q, 1: k
                            dst = qT if half == 0 else kT
                            for m in range(2):
                                for tb in range(NQB):
                                    ps = psA.tile([128, QB], F32, tag="a")
                                    for kc in range(8):
                                        nc.tensor.matmul(
                                            ps[:],
                                            wqk_sb[:, kc,
                                                   256 * half + 128 * m:
                                                   256 * half + 128 * (m + 1)],
                                            xT_sb[:, kc,
                                                  QB * tb:QB * (tb + 1)],
                                            start=(kc == 0), stop=(kc == 7))
                                    nc.scalar.add(
                                        dst[:, m, QB * tb:QB * (tb + 1)],
                                        ps[:],
                                        bqk_sb[:, 2 * half + m:2 * half + m + 1])

                        # ---------- A1 products + normalize ----------
                        prod = wl.tile([128, T], F16, tag="pr")
                        nc.vector.memset(prod[0:32, 0:1], 0.0)
                        nc.vector.memset(prod[64:96, 0:1], 0.0)
                        nc.vector.tensor_mul(
                            prod[0:32, 1:T], PAC[0:32, 0:T - 1], PBD[0:32, 1:T])
                        nc.vector.tensor_mul(
                            prod[32:64, :], PAC[32:64, :], PBD[32:64, :])
                        nc.vector.tensor_mul(
                            prod[64:96, 1:T], PAC[64:96, 0:T - 1], PBD[64:96, 1:T])
                        nc.vector.tensor_mul(
                            prod[96:128, :], PAC[96:128, :], PBD[96:128, :])
                        cds = wl.tile([64, T], F16, tag="cd")
                        nc.sync.dma_start(cds[:], prod[64:128, :])
                        u_l = wl.tile([64, T], F32, tag="ul")
                        nc.vector.tensor_sub(u_l[:], prod[0:64, :], cds[:])
                        sq = wl.tile([64, T], F16, tag="sq")
                        nc.scalar.square(sq[:], u_l[:])
                        ssq = wl.tile([64, T], F32, tag="pa")
                        for tb in range(NQB):
                            ps = psA.tile([64, QB], F32, tag="a")
                            nc.tensor.matmul(
                                ps[:], ssel[:],
                                sq[:, QB * tb:QB * (tb + 1)],
                                start=True, stop=True)
                            nc.vector.tensor_scalar_max(
                                out=ssq[:, QB * tb:QB * (tb + 1)], in0=ps[:],
                                scalar1=1e-24)
                        rt = wl.tile([64, T], F32, tag="pb")
                        nc.scalar.sqrt(rt[:], ssq[:])
                        inv = wl.tile([64, T], F32, tag="pa")
                        nc.vector.reciprocal_approx_fast(out=inv[:], in_=rt[:])
                        nc.vector.tensor_scalar_mul(
                            out=inv[:], in0=inv[:], scalar1=sbc_sb[:, 0:1])
                        u_n = wl.tile([64, T], F16, tag="sq")
                        nc.vector.tensor_mul(u_n[:], u_l[:], inv[:])
                        for p in range(NH):
                            nc.sync.dma_start(
                                out=jwT[GS[p]:GS[p] + 6, :],
                                in_=u_n[6 * p:6 * p + 6, :])
                            nc.sync.dma_start(
                                out=rlT[GS[p]:GS[p] + 6, :],
                                in_=u_n[32 + 6 * p:32 + 6 * p + 6, :])

                    # ---------- A2c gate (dense: all main MMs, then sel) ----
                    with tc.tile_pool(name="w2", bufs=4) as w2:
                        gps = []
                        for tb in range(NQB):
                            ps = psA.tile([16, QB], F32, tag="a")
                            for kc in range(8):
                                nc.tensor.matmul(
                                    ps[:], wgate_sb[:, kc, :],
                                    xT_sb[:, kc, QB * tb:QB * (tb + 1)],
                                    start=(kc == 0), stop=(kc == 7))
                            gsig = w2.tile([16, QB], F16, tag="gs")
                            nc.scalar.activation(
                                out=gsig[:], in_=ps[:],
                                func=AF.Sigmoid, bias=bgate_sb[:, 0:1], scale=1.0)
                            gps.append(gsig)

                        # ---------- A2b v|geo_v ----------
                        for ti in range(16):
                            ps = psA.tile([128, QB], F32, tag="a")
                            nc.tensor.matmul(
                                ps[:], ones_sb[:], bvg_sb[:],
                                start=True, stop=False)
                            for kc in range(8):
                                nc.tensor.matmul(
                                    ps[:],
                                    xT_sb[:, kc, 128 * ti:128 * (ti + 1)],
                                    wvg_sb[:, kc, :],
                                    start=False, stop=(kc == 7))
                            nc.vector.tensor_copy(
                                vplus[:, ti, :, 0:64],
                                ps[:, 0:256].rearrange("p (h c) -> p h c", c=64))
                            nc.vector.tensor_copy(
                                gvplus[:, ti, :, 0:64],
                                ps[:, 256:512].rearrange("p (h c) -> p h c", c=64))
                            if ti < NQB:     # gate mean, interleaved
                                psg = psA.tile([1, QB], F32, tag="g", bufs=2)
                                nc.tensor.matmul(psg[:], gsel[:], gps[ti][:],
                                                 start=True, stop=True)
                                nc.vector.tensor_copy(
                                    g_row[:, QB * ti:QB * (ti + 1)], psg[:])
                        nc.vector.tensor_scalar(
                            out=g1m_row[:], in0=g_row[:],
                            scalar1=-1.0, scalar2=1.0, op0=OP.mult, op1=OP.add)

                # xT freed; open pers3 (comb2)
                pers3 = pers3_cm.__enter__()
                comb2 = pers3.tile([128, 2, T], F16)

                # ---------- Phase B + C ----------
                with (
                    tc.tile_pool(name="pp", bufs=4) as pp,
                    tc.tile_pool(name="rows", bufs=4) as rowp,
                    tc.tile_pool(name="abt", bufs=6) as abp,
                    tc.tile_pool(name="psLs", bufs=2,
                                 space=bass.MemorySpace.PSUM) as psLs,
                    tc.tile_pool(name="psLg", bufs=2,
                                 space=bass.MemorySpace.PSUM) as psLg,
                    tc.tile_pool(name="psU", bufs=4,
                                 space=bass.MemorySpace.PSUM) as psU,
                ):
                    geo_ctr = 0

                    def emit_c(j):
                        for qt in range(4 * j, 4 * (j + 1)):
                            for et in range(2):
                                psc = psLg.tile([128, QB], F32, tag="G")
                                for hc in range(2):
                                    nc.tensor.matmul(
                                        psc[:],
                                        comb2[:, hc, 128 * qt:128 * (qt + 1)],
                                        outw_sb[:, hc, QB * et:QB * (et + 1)],
                                        start=(hc == 0), stop=(hc == 1))
                                ot = abp.tile([128, QB], F32, tag="ot", bufs=3)
                                if et == 0:
                                    nc.scalar.copy(ot[:], psc[:])
                                else:
                                    nc.vector.tensor_copy(ot[:], psc[:])
                                nc.sync.dma_start(
                                    d_partial[128 * qt:128 * (qt + 1),
                                              QB * et:QB * (et + 1)],
                                    ot[:])

                    def emit_combine(j, p, Us, Ug):
                        r, mc = p % 2, p // 2
                        qsl = slice(QB * j, QB * (j + 1))
                        sUd = rowp.tile([65, QB], F32, tag="st")
                        sUe = rowp.tile([65, QB], F32, tag="st")
                        nc.scalar.copy(sUd[64:65, :], Us[64:65, :])
                        nc.scalar.copy(sUe[64:65, :], Ug[64:65, :])
                        d0 = rowp.tile([1, QB], F32, tag="r0")
                        e0 = rowp.tile([1, QB], F32, tag="r0")
                        nc.sync.dma_start(d0[:], sUd[64:65, :])
                        nc.sync.dma_start(e0[:], sUe[64:65, :])
                        rs = rowp.tile([1, QB], F32, tag="r1")
                        rg = rowp.tile([1, QB], F32, tag="r1")
                        nc.vector.reciprocal_approx_fast(out=rs[:], in_=d0[:])
                        nc.vector.reciprocal_approx_fast(out=rg[:], in_=e0[:])
                        ar = rowp.tile([1, QB], F16, tag="r2")
                        br = rowp.tile([1, QB], F16, tag="r2")
                        nc.vector.tensor_mul(ar[:], rs[:], g1m_row[:, qsl])
                        nc.vector.tensor_mul(br[:], rg[:], g_row[:, qsl])
                        psa = psLs.tile([64, QB], F32, tag="L")
                        psb = psLg.tile([64, QB], F32, tag="G")
                        nc.tensor.matmul(psa[:], ones_sb[:, 0:64], ar[:],
                                         start=True, stop=True)
                        nc.tensor.matmul(psb[:], ones_sb[:, 0:64], br[:],
                                         start=True, stop=True)
                        aB = abp.tile([64, QB], F32, tag="ab")
                        bB = abp.tile([64, QB], F32, tag="ab")
                        nc.scalar.copy(aB[:], psa[:])
                        nc.scalar.copy(bB[:], psb[:])
                        u1 = abp.tile([64, QB], F32, tag="ab")
                        u2 = abp.tile([64, QB], F32, tag="ab")
                        nc.vector.tensor_mul(u1[:], Us[0:64, :], aB[:])
                        nc.vector.tensor_mul(u2[:], Ug[0:64, :], bB[:])
                        if r == 0:
                            nc.vector.tensor_add(
                                comb2[0:64, mc, qsl], u1[:], u2[:])
                        else:
                            ctmp = abp.tile([64, QB], F16, tag="ct", bufs=2)
                            nc.vector.tensor_add(ctmp[:], u1[:], u2[:])
                            nc.sync.dma_start(comb2[64:128, mc, qsl], ctmp[:])

                    pending = None    # (j, p, Us, Ug) awaiting combine
                    for j in range(NQB):
                        nkt = 4 * (j + 1)
                        for p in range(NH):
                            r, mc, gs = p % 2, p // 2, GS[p]
                            Us = psU.tile([65, QB], F32, tag="u", name="Us")
                            Ug = psU.tile([65, QB], F32, tag="u", name="Ug")
                            for kt in range(nkt):
                                m = kt - 4 * j
                                c0 = min(128 * m, 256) if m >= 1 else 0
                                ksl = slice(KT * kt, KT * (kt + 1))
                                qsl = slice(QB * j + c0, QB * (j + 1))
                                Ls = psLs.tile([128, QB], F32, tag="L")
                                Lg = psLg.tile([128, QB], F32, tag="G")
                                nc.tensor.matmul(
                                    Ls[:, c0:], kT[64 * r:64 * r + 64, mc, ksl],
                                    qT[64 * r:64 * r + 64, mc, qsl],
                                    start=True, stop=True,
                                    tile_position=(64 * r, 0))
                                nc.tensor.matmul(
                                    Lg[:, c0:], jwT[gs:gs + 6, ksl],
                                    rlT[gs:gs + 6, qsl],
                                    start=True, stop=True,
                                    tile_position=(gs, 0))
                                if kt == 1 and pending is not None:
                                    emit_combine(*pending)
                                    pending = None
                                if kt == 2 and p == 1 and j > 0:
                                    emit_c(j - 1)
                                Ps = pp.tile([128, QB], F16, tag="P")
                                Pg = pp.tile([128, QB], I16, tag="Q")
                                nc.scalar.activation(Ps[:, c0:], Ls[:, c0:],
                                                     AF.Exp)
                                geo_ctr += 1
                                if geo_ctr % GEO_ACT_EVERY == 0:
                                    nc.scalar.activation(
                                        Pg[:, c0:].bitcast(F16),
                                        Lg[:, c0:], AF.Exp)
                                else:
                                    nc.vector.tensor_scalar(
                                        out=Pg[:, c0:], in0=Lg[:, c0:],
                                        scalar1=A_SCH, scalar2=B_SCH,
                                        op0=OP.mult, op1=OP.add)
                                if m >= 0:
                                    w = KT * (m + 1)
                                    for Px in (Ps[:, c0:w],
                                               Pg[:, c0:w].bitcast(F16)):
                                        nc.gpsimd.affine_select(
                                            out=Px, in_=Px,
                                            compare_op=OP.is_ge, fill=0.0,
                                            base=c0 - KT * m,
                                            pattern=[[1, w - c0]],
                                            channel_multiplier=-1)
                                nc.tensor.matmul(
                                    Us[:, c0:],
                                    vplus[:, kt, p, :], Ps[:, c0:],
                                    start=(kt == 0), stop=(kt == nkt - 1))
                                nc.tensor.matmul(
                                    Ug[:, c0:],
                                    gvplus[:, kt, p, :],
                                    Pg[:, c0:].bitcast(F16),
                                    start=(kt == 0), stop=(kt == nkt - 1))
                            pending = (j, p, Us, Ug)
                    emit_combine(*pending)
                    emit_c(NQB - 1)
            finally:
                if pers3 is not None:
                    pers3_cm.__exit__(None, None, None)
                pers2_cm.__exit__(None, None, None)
    nc.compile()
    return nc


_nc_cache = None


def _get_nc():
    global _nc_cache
    if _nc_cache is None:
        _nc_cache = _build_nc()
    return _nc_cache


def _prep_core_inputs(inputs, core):
    b = core // 4
    h0 = (core % 4) * 4
    f = np.float32
    qkv_w, qkv_b = inputs['qkv_w'], inputs['qkv_b']
    scale = DH ** -0.5
    s = slice(h0 * DH, h0 * DH + NH * DH)
    ac = np.ascontiguousarray

    # Lines operand layout (cols of each [D,64] half): rows 0:24 write-path
    # (+pad8), rows 32:56 read-path (+pad8). A/C get the x_prev shift side.
    WLA = np.zeros((D, 64), f); WLB = np.zeros((D, 64), f)
    WLC = np.zeros((D, 64), f); WLD = np.zeros((D, 64), f)
    w1w, w2w = inputs['w1_write'], inputs['w2_write']
    w1r, w2r = inputs['w1_read'], inputs['w2_read']
    for h in range(NH):
        gh = h0 + h
        for jj in range(6):
            i_, j_ = PAIRS4[5 - jj]
            WLA[:, 0 + h * 6 + jj] = w1w[:, gh * 4 + i_] * SIGMA[jj]    # A_w
            WLB[:, 0 + h * 6 + jj] = w2w[:, gh * 4 + j_]                # B_w
            WLC[:, 0 + h * 6 + jj] = w1w[:, gh * 4 + j_] * SIGMA[jj]    # C_w
            WLD[:, 0 + h * 6 + jj] = w2w[:, gh * 4 + i_]                # D_w
        for pp in range(6):
            i_, j_ = PAIRS4[pp]
            WLA[:, 32 + h * 6 + pp] = w1r[:, gh * 4 + i_]               # A_r
            WLB[:, 32 + h * 6 + pp] = w2r[:, gh * 4 + j_]               # B_r
            WLC[:, 32 + h * 6 + pp] = w1r[:, gh * 4 + j_]               # C_r
            WLD[:, 32 + h * 6 + pp] = w2r[:, gh * 4 + i_]               # D_r

    ssel = np.zeros((64, 64), f)
    for half in (0, 32):
        for h in range(NH):
            g = slice(half + 6 * h, half + 6 * h + 6)
            ssel[g, g] = 1.0
    sbc = np.ones((64, 1), f)
    sbc[32:56, 0] = np.repeat(inputs['inc_scale'][h0:h0 + NH], 6).astype(f)

    wq = (qkv_w[:, 0 * D:1 * D][:, s] * scale).astype(f)
    wk = qkv_w[:, 1 * D:2 * D][:, s].astype(f)
    bq = (qkv_b[0 * D:1 * D][s] * scale).astype(f)
    bk = qkv_b[1 * D:2 * D][s].astype(f)
    bqk = np.stack([bq[0:128], bq[128:256], bk[0:128], bk[128:256]], axis=1)

    outw = inputs['out_w'][s, :].astype(f)

    def chunked(w):
        # [D, C] -> [128, 8*C] with row p holding chunks [k, p, :] contiguous
        C = w.shape[1]
        return ac(w.reshape(8, 128, C).transpose(1, 0, 2).reshape(128, 8 * C)
                  .astype(F16NP))

    return {
        'xT': ac(np.asarray(inputs['x'][b], f).T.astype(F16NP)),
        'wqk': chunked(np.concatenate([wq, wk], axis=1)),
        'bqk': ac(bqk),
        'wvg': chunked(np.concatenate(
            [qkv_w[:, 2 * D:3 * D][:, s], inputs['geo_w'][:, s]],
            axis=1).astype(f)),
        'bvg': ac(np.concatenate(
            [qkv_b[2 * D:3 * D][s], inputs['geo_b'][s]]).astype(F16NP)
            .reshape(1, 512)),
        'wlAC': chunked(np.concatenate([WLA, WLC], axis=1)),
        'wlBD': chunked(np.concatenate([WLB, WLD], axis=1)),
        'wgate': chunked(inputs['gate_w'].astype(f)),
        'bgate': ac(inputs['gate_b'].astype(f).reshape(16, 1)),
        'gsel': np.full((16, 1), 1.0 / 16.0, F16NP),
        'sbc': sbc,
        'ssel': ssel.astype(F16NP),
        'ones': np.ones((1, 128), F16NP),
        'outw': ac(outw.astype(F16NP).reshape(2, 128, D).transpose(1, 0, 2)
                   .reshape(128, 2 * D)),
    }


def kernel(**inputs):
    global LAST_RESULT
    inputs = {k: np.asarray(v) for k, v in inputs.items()}
    nc = _get_nc()
    in_maps = [_prep_core_inputs(inputs, c) for c in range(NCORES)]
    res = run_bass_kernel_spmd(nc, in_maps, core_ids=list(range(NCORES)),
                               trace=TRACE)
    LAST_RESULT = res
    out = np.zeros((B, T, D), np.float32)
    for c in range(NCORES):
        out[c // 4] += res.results[c]['partial']
    out += np.asarray(inputs['out_b'], np.float32)[None, None, :]
    return out


# revision 16
# speedup vs baseline: 1.2959x; 1.2959x over previous
"""DualPathAttention Trainium2 Bass kernel (v4, fp16 + software pipelining).

Sharding: batch*head parallel across 8 cores. Core c handles batch b=c//4 and
global heads [4*(c%4), 4*(c%4)+4). Each core computes its 4 heads' dual-path
attention and the partial final projection (its 256 rows of out_w); the host
sums the 4 partials per batch and adds out_b.

Per-core structure:
  A1  Pluecker lines: packed projections PAC=[A|C]^T x, PBD=[B|D]^T x (the
      gate weights ride along unused rows), products with the x_prev shift,
      CD half shifted down via SBUF DMA, normalize, scatter per-head slots.
  A2a q^T/k^T as head pairs [128, 2, T] (no padding); std logits K=64.
  A2b v|geo_v in one [T x 512] stream into the U-matmul lhsT layout with a
      ones column (softmax denominators at PSUM row 64 for free).
  B   per (q-block j, head p): nkt std-logits (K=64) + geo-logits (K=6)
      matmuls, exp split ACT/DVE (geo via fp16 Schraudolph int-bitcast),
      causal masks on gpsimd, U accumulation; block-diagonal tiles compute
      only columns >= min(128*m, 256). Combine (alpha=(1-g)/Dstd etc. via
      K=1 ones-broadcast matmuls) is emitted one head late, and phase C one
      q-block late, so the in-order PE queue never stalls on their deps.
  C   comb2 [128, 2, T] packs head pairs: final projection contracts K=128
      with no padding (odd heads reach rows 64:128 via small SBUF DMA).
All matmul storage is fp16 (1 cyc/row like bf16 but ~8x lower rounding
error; every value here is < 6e4 so range is fine). PSUM stays fp32.
Weights are host-pre-arranged to contiguous [128, 8*C] chunk layout and all
weight DMAs issue upfront, overlapped with A1.
"""

import os
import numpy as np

import concourse.bass as bass
from concourse import bacc
import concourse.mybir as mybir
import concourse.tile as tile
from concourse.bass_utils import run_bass_kernel_spmd

D, H, B, T = 1024, 16, 2, 2048
DH = 64          # head dim
NH = 4           # heads per core
NCORES = 8
QB = 512         # q block width
KT = 128         # k tile height
NQB = T // QB    # 4
F32 = mybir.dt.float32
I16 = mybir.dt.int16
F16 = mybir.dt.float16

PAIRS4 = [(0, 1), (0, 2), (0, 3), (1, 2), (1, 3), (2, 3)]
SIGMA = [1.0, -1.0, 1.0, 1.0, -1.0, 1.0]
GS = [64, 0, 96, 32]      # geo line row-slot per local head (opposite half)
GEO_ACT_EVERY = 2         # every Nth geo exp tile runs on ACT instead of DVE
A_SCH = 1477.3197218702985   # 2^10/ln2  (fp16-bitcast Schraudolph exp)
B_SCH = 15315.0              # (15<<10) - 45: centered sawtooth

TRACE = False            # set by test harness for profiling runs
LAST_RESULT = None       # BassKernelResults of last run (for exec_time_ns)

F16NP = np.float16


def _build_nc():
    nc = bacc.Bacc("TRN2", target_bir_lowering=False, debug=False)

    # ---- DRAM I/O ----
    d_xT = nc.dram_tensor("xT", [D, T], F16, kind="ExternalInput")
    d_wqk = nc.dram_tensor("wqk", [128, 4096], F16, kind="ExternalInput")
    d_bqk = nc.dram_tensor("bqk", [128, 4], F32, kind="ExternalInput")
    d_wvg = nc.dram_tensor("wvg", [128, 4096], F16, kind="ExternalInput")
    d_bvg = nc.dram_tensor("bvg", [1, 512], F16, kind="ExternalInput")
    d_wlAC = nc.dram_tensor("wlAC", [128, 1024], F16, kind="ExternalInput")
    d_wlBD = nc.dram_tensor("wlBD", [128, 1024], F16, kind="ExternalInput")
    d_wgate = nc.dram_tensor("wgate", [128, 128], F16, kind="ExternalInput")
    d_bgate = nc.dram_tensor("bgate", [16, 1], F32, kind="ExternalInput")
    d_gsel = nc.dram_tensor("gsel", [16, 1], F16, kind="ExternalInput")
    d_sbc = nc.dram_tensor("sbc", [64, 1], F32, kind="ExternalInput")
    d_ssel = nc.dram_tensor("ssel", [64, 64], F16, kind="ExternalInput")
    d_ones = nc.dram_tensor("ones", [1, 128], F16, kind="ExternalInput")
    d_outw = nc.dram_tensor("outw", [128, 2 * D], F16, kind="ExternalInput")
    d_partial = nc.dram_tensor("partial", [T, D], F32, kind="ExternalOutput")

    AF = mybir.ActivationFunctionType
    OP = mybir.AluOpType

    with tile.TileContext(nc, linearize=bool(int(os.environ.get('KLIN', '0')))) as tc:
        with (
            tc.tile_pool(name="const", bufs=1) as cpool,
            tc.tile_pool(name="pers1", bufs=1) as pers1,
        ):
            # ---- constants ----
            ones_sb = cpool.tile([1, 128], F16)
            nc.sync.dma_start(ones_sb[:], d_ones[:])
            gsel = cpool.tile([16, 1], F16)
            nc.sync.dma_start(gsel[:], d_gsel[:])
            ssel = cpool.tile([64, 64], F16)
            nc.sync.dma_start(ssel[:], d_ssel[:])
            sbc_sb = cpool.tile([64, 1], F32)
            nc.sync.dma_start(sbc_sb[:], d_sbc[:])
            bgate_sb = cpool.tile([16, 1], F32)
            nc.sync.dma_start(bgate_sb[:], d_bgate[:])

            jwT = pers1.tile([128, T], F16)   # head p lines at rows GS[p]:+6
            rlT = pers1.tile([128, T], F16)
            g_row = pers1.tile([1, T], F32)
            g1m_row = pers1.tile([1, T], F32)
            outw_sb = pers1.tile([128, 2, D], F16)

            pers2_cm = tc.tile_pool(name="pers2", bufs=1, side="right")
            pers3_cm = tc.tile_pool(name="pers3", bufs=1, side="right")
            pers2 = pers2_cm.__enter__()
            pers3 = None
            try:
                qT = pers2.tile([128, 2, T], F16)
                kT = pers2.tile([128, 2, T], F16)
                vplus = pers2.tile([128, 16, 4, 65], F16)
                gvplus = pers2.tile([128, 16, 4, 65], F16)
                nc.vector.memset(vplus[:, :, :, 64:65], 1.0)
                nc.vector.memset(gvplus[:, :, :, 64:65], 1.0)

                with (
                    tc.tile_pool(name="wA", bufs=1) as wA,
                    tc.tile_pool(name="xp", bufs=1) as xp,
                    tc.tile_pool(name="psA", bufs=4,
                                 space=bass.MemorySpace.PSUM) as psA,
                ):
                    xT_sb = xp.tile([128, 8, T], F16)
                    # all weights upfront, contiguous rows (host pre-arranged)
                    wlAC_sb = wA.tile([128, 8, 128], F16)
                    wlBD_sb = wA.tile([128, 8, 128], F16)
                    wqk_sb = wA.tile([128, 8, 512], F16)
                    wvg_sb = wA.tile([128, 8, 512], F16)
                    wgate_sb = wA.tile([128, 8, 16], F16)
                    bqk_sb = wA.tile([128, 4], F32)
                    bvg_sb = wA.tile([1, 512], F16)
                    flat = "p a b -> p (a b)"
                    nc.sync.dma_start(wlAC_sb[:].rearrange(flat), d_wlAC[:])
                    nc.sync.dma_start(wlBD_sb[:].rearrange(flat), d_wlBD[:])
                    nc.sync.dma_start(xT_sb[:, 0, :], d_xT[0:128, :])
                    nc.sync.dma_start(xT_sb[:, 1, :], d_xT[128:256, :])
                    nc.sync.dma_start(wqk_sb[:].rearrange(flat), d_wqk[:])
                    for ko in range(2, 8):
                        nc.sync.dma_start(
                            xT_sb[:, ko, :], d_xT[128 * ko:128 * (ko + 1), :])
                    nc.sync.dma_start(wvg_sb[:].rearrange(flat), d_wvg[:])
                    nc.sync.dma_start(wgate_sb[:].rearrange(flat), d_wgate[:])
                    nc.sync.dma_start(bqk_sb[:], d_bqk[:])
                    nc.sync.dma_start(bvg_sb[:], d_bvg[:])
                    nc.sync.dma_start(outw_sb[:].rearrange(flat), d_outw[:])

                    with (
                        tc.tile_pool(name="wl", bufs=1) as wl,
                        tc.tile_pool(name="w2", bufs=4) as w2,
                    ):
                        # ---------- A1 projections ----------
                        PAC = wl.tile([128, T], F16, tag="pa")
                        PBD = wl.tile([128, T], F16, tag="pb")
                        for (dst, wsb) in ((PAC, wlAC_sb), (PBD, wlBD_sb)):
                            for tb in range(NQB):
                                ps = psA.tile([128, QB], F32, tag="a")
                                for kc in range(8):
                                    nc.tensor.matmul(
                                        ps[:], wsb[:, kc, :],
                                        xT_sb[:, kc, QB * tb:QB * (tb + 1)],
                                        start=(kc == 0), stop=(kc == 7))
                                nc.scalar.copy(dst[:, QB * tb:QB * (tb + 1)],
                                               ps[:])

                        # ---------- gate mains (sigmoids run early) ----------
                        gps = []
                        for tb in range(NQB):
                            ps = psA.tile([16, QB], F32, tag="a")
                            for kc in range(8):
                                nc.tensor.matmul(
                                    ps[:], wgate_sb[:, kc, :],
                                    xT_sb[:, kc, QB * tb:QB * (tb + 1)],
                                    start=(kc == 0), stop=(kc == 7))
                            gsig = w2.tile([16, QB], F16, tag="gs")
                            nc.scalar.activation(
                                out=gsig[:], in_=ps[:],
                                func=AF.Sigmoid, bias=bgate_sb[:, 0:1], scale=1.0)
                            gps.append(gsig)

                        # ---------- A1 products (DVE/ACT; overlaps A2a) -----
                        prod = wl.tile([128, T], F16, tag="pr")
                        nc.vector.memset(prod[0:32, 0:1], 0.0)
                        nc.vector.memset(prod[64:96, 0:1], 0.0)
                        nc.vector.tensor_mul(
                            prod[0:32, 1:T], PAC[0:32, 0:T - 1], PBD[0:32, 1:T])
                        nc.vector.tensor_mul(
                            prod[32:64, :], PAC[32:64, :], PBD[32:64, :])
                        nc.vector.tensor_mul(
                            prod[64:96, 1:T], PAC[64:96, 0:T - 1], PBD[64:96, 1:T])
                        nc.vector.tensor_mul(
                            prod[96:128, :], PAC[96:128, :], PBD[96:128, :])
                        cds = wl.tile([64, T], F16, tag="cd")
                        nc.gpsimd.dma_start(cds[:], prod[64:128, :])
                        u_l = wl.tile([64, T], F32, tag="ul")
                        nc.vector.tensor_sub(u_l[:], prod[0:64, :], cds[:])
                        sq = wl.tile([64, T], F16, tag="sq")
                        nc.scalar.square(sq[:], u_l[:])

                        # ---------- A2a q/k ----------
                        for half in range(2):           # 0: q, 1: k
                            dst = qT if half == 0 else kT
                            for m in range(2):
                                for tb in range(NQB):
                                    ps = psA.tile([128, QB], F32, tag="a")
                                    for kc in range(8):
                                        nc.tensor.matmul(
                                            ps[:],
                                            wqk_sb[:, kc,
                                                   256 * half + 128 * m:
                                                   256 * half + 128 * (m + 1)],
                                            xT_sb[:, kc,
                                                  QB * tb:QB * (tb + 1)],
                                            start=(kc == 0), stop=(kc == 7))
                                    nc.scalar.add(
                                        dst[:, m, QB * tb:QB * (tb + 1)],
                                        ps[:],
                                        bqk_sb[:, 2 * half + m:2 * half + m + 1])

                        # ---------- lines normalize + scatter ----------
                        ssq = wl.tile([64, T], F32, tag="pa")
                        for tb in range(NQB):
                            ps = psA.tile([64, QB], F32, tag="a")
                            nc.tensor.matmul(
                                ps[:], ssel[:],
                                sq[:, QB * tb:QB * (tb + 1)],
                                start=True, stop=True)
                            nc.vector.tensor_scalar_max(
                                out=ssq[:, QB * tb:QB * (tb + 1)], in0=ps[:],
                                scalar1=1e-24)
                        rt = wl.tile([64, T], F32, tag="pb")
                        nc.scalar.sqrt(rt[:], ssq[:])
                        inv = wl.tile([64, T], F32, tag="pa")
                        nc.vector.reciprocal_approx_fast(out=inv[:], in_=rt[:])
                        nc.vector.tensor_scalar_mul(
                            out=inv[:], in0=inv[:], scalar1=sbc_sb[:, 0:1])
                        u_n = wl.tile([64, T], F16, tag="sq")
                        nc.vector.tensor_mul(u_n[:], u_l[:], inv[:])
                        for p in range(NH):
                            nc.gpsimd.dma_start(
                                out=jwT[GS[p]:GS[p] + 6, :],
                                in_=u_n[6 * p:6 * p + 6, :])
                            nc.gpsimd.dma_start(
                                out=rlT[GS[p]:GS[p] + 6, :],
                                in_=u_n[32 + 6 * p:32 + 6 * p + 6, :])

                        # ---------- A2b v|geo_v (+ gate mean) ----------
                        for ti in range(16):
                            ps = psA.tile([128, QB], F32, tag="a")
                            nc.tensor.matmul(
                                ps[:], ones_sb[:], bvg_sb[:],
                                start=True, stop=False)
                            for kc in range(8):
                                nc.tensor.matmul(
                                    ps[:],
                                    xT_sb[:, kc, 128 * ti:128 * (ti + 1)],
                                    wvg_sb[:, kc, :],
                                    start=False, stop=(kc == 7))
                            nc.vector.tensor_copy(
                                vplus[:, ti, :, 0:64],
                                ps[:, 0:256].rearrange("p (h c) -> p h c", c=64))
                            nc.vector.tensor_copy(
                                gvplus[:, ti, :, 0:64],
                                ps[:, 256:512].rearrange("p (h c) -> p h c", c=64))
                            if ti < NQB:     # gate mean, interleaved
                                psg = psA.tile([1, QB], F32, tag="g", bufs=2)
                                nc.tensor.matmul(psg[:], gsel[:], gps[ti][:],
                                                 start=True, stop=True)
                                nc.vector.tensor_copy(
                                    g_row[:, QB * ti:QB * (ti + 1)], psg[:])
                        nc.vector.tensor_scalar(
                            out=g1m_row[:], in0=g_row[:],
                            scalar1=-1.0, scalar2=1.0, op0=OP.mult, op1=OP.add)

                # xT freed; open pers3 (comb2)
                pers3 = pers3_cm.__enter__()
                comb2 = pers3.tile([128, 2, T], F16)

                # ---------- Phase B + C ----------
                with (
                    tc.tile_pool(name="pp", bufs=4) as pp,
                    tc.tile_pool(name="rows", bufs=4) as rowp,
                    tc.tile_pool(name="abt", bufs=6) as abp,
                    tc.tile_pool(name="psLs", bufs=2,
                                 space=bass.MemorySpace.PSUM) as psLs,
                    tc.tile_pool(name="psLg", bufs=2,
                                 space=bass.MemorySpace.PSUM) as psLg,
                    tc.tile_pool(name="psU", bufs=4,
                                 space=bass.MemorySpace.PSUM) as psU,
                ):
                    geo_ctr = 0

                    def emit_c(j):
                        for qt in range(4 * j, 4 * (j + 1)):
                            for et in range(2):
                                psc = psLg.tile([128, QB], F32, tag="G")
                                for hc in range(2):
                                    nc.tensor.matmul(
                                        psc[:],
                                        comb2[:, hc, 128 * qt:128 * (qt + 1)],
                                        outw_sb[:, hc, QB * et:QB * (et + 1)],
                                        start=(hc == 0), stop=(hc == 1))
                                ot = abp.tile([128, QB], F32, tag="ot", bufs=3)
                                if et == 0:
                                    nc.scalar.copy(ot[:], psc[:])
                                else:
                                    nc.vector.tensor_copy(ot[:], psc[:])
                                nc.sync.dma_start(
                                    d_partial[128 * qt:128 * (qt + 1),
                                              QB * et:QB * (et + 1)],
                                    ot[:])

                    def emit_combine(j, p, Us, Ug):
                        r, mc = p % 2, p // 2
                        qsl = slice(QB * j, QB * (j + 1))
                        sUd = rowp.tile([65, QB], F32, tag="st")
                        sUe = rowp.tile([65, QB], F32, tag="st")
                        nc.scalar.copy(sUd[64:65, :], Us[64:65, :])
                        nc.scalar.copy(sUe[64:65, :], Ug[64:65, :])
                        d0 = rowp.tile([1, QB], F32, tag="r0")
                        e0 = rowp.tile([1, QB], F32, tag="r0")
                        nc.sync.dma_start(d0[:], sUd[64:65, :])
                        nc.sync.dma_start(e0[:], sUe[64:65, :])
                        rs = rowp.tile([1, QB], F32, tag="r1")
                        rg = rowp.tile([1, QB], F32, tag="r1")
                        nc.vector.reciprocal_approx_fast(out=rs[:], in_=d0[:])
                        nc.vector.reciprocal_approx_fast(out=rg[:], in_=e0[:])
                        ar = rowp.tile([1, QB], F16, tag="r2")
                        br = rowp.tile([1, QB], F16, tag="r2")
                        nc.vector.tensor_mul(ar[:], rs[:], g1m_row[:, qsl])
                        nc.vector.tensor_mul(br[:], rg[:], g_row[:, qsl])
                        psa = psLs.tile([64, QB], F32, tag="L")
                        psb = psLg.tile([64, QB], F32, tag="G")
                        nc.tensor.matmul(psa[:], ones_sb[:, 0:64], ar[:],
                                         start=True, stop=True)
                        nc.tensor.matmul(psb[:], ones_sb[:, 0:64], br[:],
                                         start=True, stop=True)
                        aB = abp.tile([64, QB], F32, tag="ab")
                        bB = abp.tile([64, QB], F32, tag="ab")
                        nc.scalar.copy(aB[:], psa[:])
                        nc.scalar.copy(bB[:], psb[:])
                        u1 = abp.tile([64, QB], F32, tag="ab")
                        u2 = abp.tile([64, QB], F32, tag="ab")
                        nc.vector.tensor_mul(u1[:], Us[0:64, :], aB[:])
                        nc.vector.tensor_mul(u2[:], Ug[0:64, :], bB[:])
                        if r == 0:
                            nc.vector.tensor_add(
                                comb2[0:64, mc, qsl], u1[:], u2[:])
                        else:
                            ctmp = abp.tile([64, QB], F16, tag="ct", bufs=2)
                            nc.vector.tensor_add(ctmp[:], u1[:], u2[:])
                            nc.sync.dma_start(comb2[64:128, mc, qsl], ctmp[:])

                    pending = None    # (j, p, Us, Ug) awaiting combine
                    for j in range(NQB):
                        nkt = 4 * (j + 1)
                        for p in range(NH):
                            r, mc, gs = p % 2, p // 2, GS[p]
                            Us = psU.tile([65, QB], F32, tag="u", name="Us")
                            Ug = psU.tile([65, QB], F32, tag="u", name="Ug")
                            for kt in range(nkt):
                                m = kt - 4 * j
                                c0 = min(128 * m, 256) if m >= 1 else 0
                                ksl = slice(KT * kt, KT * (kt + 1))
                                qsl = slice(QB * j + c0, QB * (j + 1))
                                Ls = psLs.tile([128, QB], F32, tag="L")
                                Lg = psLg.tile([128, QB], F32, tag="G")
                                nc.tensor.matmul(
                                    Ls[:, c0:], kT[64 * r:64 * r + 64, mc, ksl],
                                    qT[64 * r:64 * r + 64, mc, qsl],
                                    start=True, stop=True,
                                    tile_position=(64 * r, 0))
                                nc.tensor.matmul(
                                    Lg[:, c0:], jwT[gs:gs + 6, ksl],
                                    rlT[gs:gs + 6, qsl],
                                    start=True, stop=True,
                                    tile_position=(gs, 0))
                                if kt == 3 and pending is not None:
                                    emit_combine(*pending)
                                    pending = None
                                if kt == 2 and p == 2 and j > 0:
                                    emit_c(j - 1)
                                Ps = pp.tile([128, QB], F16, tag="P")
                                Pg = pp.tile([128, QB], I16, tag="Q")
                                nc.scalar.activation(Ps[:, c0:], Ls[:, c0:],
                                                     AF.Exp)
                                geo_ctr += 1
                                if geo_ctr % GEO_ACT_EVERY == 0:
                                    nc.scalar.activation(
                                        Pg[:, c0:].bitcast(F16),
                                        Lg[:, c0:], AF.Exp)
                                else:
                                    nc.vector.tensor_scalar(
                                        out=Pg[:, c0:], in0=Lg[:, c0:],
                                        scalar1=A_SCH, scalar2=B_SCH,
                                        op0=OP.mult, op1=OP.add)
                                if m >= 0:
                                    w = KT * (m + 1)
                                    for Px in (Ps[:, c0:w],
                                               Pg[:, c0:w].bitcast(F16)):
                                        nc.gpsimd.affine_select(
                                            out=Px, in_=Px,
                                            compare_op=OP.is_ge, fill=0.0,
                                            base=c0 - KT * m,
                                            pattern=[[1, w - c0]],
                                            channel_multiplier=-1)
                                nc.tensor.matmul(
                                    Us[:, c0:],
                                    vplus[:, kt, p, :], Ps[:, c0:],
                                    start=(kt == 0), stop=(kt == nkt - 1))
                                nc.tensor.matmul(
                                    Ug[:, c0:],
                                    gvplus[:, kt, p, :],
                                    Pg[:, c0:].bitcast(F16),
                                    start=(kt == 0), stop=(kt == nkt - 1))
                            pending = (j, p, Us, Ug)
                    emit_combine(*pending)
                    emit_c(NQB - 1)
            finally:
                if pers3 is not None:
                    pers3_cm.__exit__(None, None, None)
                pers2_cm.__exit__(None, None, None)
    nc.compile()
    return nc


_nc_cache = None


def _get_nc():
    global _nc_cache
    if _nc_cache is None:
        _nc_cache = _build_nc()
    return _nc_cache


def _prep_core_inputs(inputs, core):
    b = core // 4
    h0 = (core % 4) * 4
    f = np.float32
    qkv_w, qkv_b = inputs['qkv_w'], inputs['qkv_b']
    scale = DH ** -0.5
    s = slice(h0 * DH, h0 * DH + NH * DH)
    ac = np.ascontiguousarray

    # Lines operand layout (cols of each [D,64] half): rows 0:24 write-path
    # (+pad8), rows 32:56 read-path (+pad8). A/C get the x_prev shift side.
    WLA = np.zeros((D, 64), f); WLB = np.zeros((D, 64), f)
    WLC = np.zeros((D, 64), f); WLD = np.zeros((D, 64), f)
    w1w, w2w = inputs['w1_write'], inputs['w2_write']
    w1r, w2r = inputs['w1_read'], inputs['w2_read']
    for h in range(NH):
        gh = h0 + h
        for jj in range(6):
            i_, j_ = PAIRS4[5 - jj]
            WLA[:, 0 + h * 6 + jj] = w1w[:, gh * 4 + i_] * SIGMA[jj]    # A_w
            WLB[:, 0 + h * 6 + jj] = w2w[:, gh * 4 + j_]                # B_w
            WLC[:, 0 + h * 6 + jj] = w1w[:, gh * 4 + j_] * SIGMA[jj]    # C_w
            WLD[:, 0 + h * 6 + jj] = w2w[:, gh * 4 + i_]                # D_w
        for pp in range(6):
            i_, j_ = PAIRS4[pp]
            WLA[:, 32 + h * 6 + pp] = w1r[:, gh * 4 + i_]               # A_r
            WLB[:, 32 + h * 6 + pp] = w2r[:, gh * 4 + j_]               # B_r
            WLC[:, 32 + h * 6 + pp] = w1r[:, gh * 4 + j_]               # C_r
            WLD[:, 32 + h * 6 + pp] = w2r[:, gh * 4 + i_]               # D_r

    ssel = np.zeros((64, 64), f)
    for half in (0, 32):
        for h in range(NH):
            g = slice(half + 6 * h, half + 6 * h + 6)
            ssel[g, g] = 1.0
    sbc = np.ones((64, 1), f)
    sbc[32:56, 0] = np.repeat(inputs['inc_scale'][h0:h0 + NH], 6).astype(f)

    wq = (qkv_w[:, 0 * D:1 * D][:, s] * scale).astype(f)
    wk = qkv_w[:, 1 * D:2 * D][:, s].astype(f)
    bq = (qkv_b[0 * D:1 * D][s] * scale).astype(f)
    bk = qkv_b[1 * D:2 * D][s].astype(f)
    bqk = np.stack([bq[0:128], bq[128:256], bk[0:128], bk[128:256]], axis=1)

    outw = inputs['out_w'][s, :].astype(f)

    def chunked(w):
        # [D, C] -> [128, 8*C] with row p holding chunks [k, p, :] contiguous
        C = w.shape[1]
        return ac(w.reshape(8, 128, C).transpose(1, 0, 2).reshape(128, 8 * C)
                  .astype(F16NP))

    return {
        'xT': ac(np.asarray(inputs['x'][b], f).T.astype(F16NP)),
        'wqk': chunked(np.concatenate([wq, wk], axis=1)),
        'bqk': ac(bqk),
        'wvg': chunked(np.concatenate(
            [qkv_w[:, 2 * D:3 * D][:, s], inputs['geo_w'][:, s]],
            axis=1).astype(f)),
        'bvg': ac(np.concatenate(
            [qkv_b[2 * D:3 * D][s], inputs['geo_b'][s]]).astype(F16NP)
            .reshape(1, 512)),
        'wlAC': chunked(np.concatenate([WLA, WLC], axis=1)),
        'wlBD': chunked(np.concatenate([WLB, WLD], axis=1)),
        'wgate': chunked(inputs['gate_w'].astype(f)),
        'bgate': ac(inputs['gate_b'].astype(f).reshape(16, 1)),
        'gsel': np.full((16, 1), 1.0 / 16.0, F16NP),
        'sbc': sbc,
        'ssel': ssel.astype(F16NP),
        'ones': np.ones((1, 128), F16NP),
        'outw': ac(outw.astype(F16NP).reshape(2, 128, D).transpose(1, 0, 2)
                   .reshape(128, 2 * D)),
    }


def kernel(**inputs):
    global LAST_RESULT
    inputs = {k: np.asarray(v) for k, v in inputs.items()}
    nc = _get_nc()
    in_maps = [_prep_core_inputs(inputs, c) for c in range(NCORES)]
    res = run_bass_kernel_spmd(nc, in_maps, core_ids=list(range(NCORES)),
                               trace=TRACE)
    LAST_RESULT = res
    out = np.zeros((B, T, D), np.float32)
    for c in range(NCORES):
        out[c // 4] += res.results[c]['partial']
    out += np.asarray(inputs['out_b'], np.float32)[None, None, :]
    return out
